# revision 1
# baseline (speedup 1.0000x reference)
"""Trainium2 Bass kernel for nn_DeepManualLSTM (3-layer LSTM, B=1024, T=48, IN=64, H=512).

Data-parallel over batch (128 rows/core x 8 cores); weights SBUF-resident.
All matmul inputs in bf16 (measured 204.6 ns/MM at N=512 vs 296-362 ns for
fp32r whose LDWEIGHTS serializes; end-to-end rel err ~4e-3, tolerance 2e-2).
Orientation: activations transposed (feature-major) stationary, weights
moving, N=512 per PSUM bank; k-outer ordering reuses each stationary across
the 4 gate banks. Gate columns are host-reordered to [f,i,o,c] so one
sigmoid ACT covers 1536 contiguous columns and one tanh covers 512.
Gates/C/h all bf16 => DVE 2x_1P mode on the elementwise tail; h re-enters
the matmuls via 4 bf16 PE transposes (75 ns each) + one DVE copy.
The 48-step recurrence is fully unrolled in wavefront order (cells (s,0),
(s-1,1), (s-2,2)) so each cell's tail hides under other cells' matmuls.
Final [B,H]@[H,1] projection runs on the host.
"""
import sys
import os

for _p in ("/opt/trn_rl_repo", "/root/.axon_site/_ro/trn_rl_repo"):
    if os.path.isdir(_p) and _p not in sys.path:
        sys.path.insert(0, _p)

import numpy as np
import ml_dtypes

import concourse.bass as bass
import concourse.tile as tile
from concourse import bacc, mybir
from concourse import bass_utils
from concourse.bass import ds, ts
from concourse.masks import make_identity

P = 128          # batch rows per core / SBUF partitions
T = 48           # sequence length
IN = 64          # input features
H = 512          # hidden size
L = 3            # layers
G4 = 4 * H       # gate width (2048)
NB = 4           # PSUM banks per gate row (G4 / 512)
KH = H // P      # k-chunks of the hidden contraction (4)
NCORES = 8

F32 = mybir.dt.float32
BF16 = mybir.dt.bfloat16
AF = mybir.ActivationFunctionType


def _to_bf16(a: np.ndarray) -> np.ndarray:
    """fp32 -> bf16 (round-to-nearest-even)."""
    return np.ascontiguousarray(a, dtype=np.float32).astype(ml_dtypes.bfloat16)


def _build(include_bias: bool, reps: int = 1) -> bass.Bass:
    nc = bacc.Bacc()

    # x pre-transposed on the host: [128, (T//2)*128] bf16, even t in
    # partitions 0:64, odd t in 64:128, column block t//2 holds x_t^T.
    xT_d = nc.dram_tensor("xT", [P, (T // 2) * P], BF16, kind="ExternalInput")
    wx_d = [
        nc.dram_tensor("wx0", [IN, G4], BF16, kind="ExternalInput"),
        nc.dram_tensor("wx1", [H, G4], BF16, kind="ExternalInput"),
        nc.dram_tensor("wx2", [H, G4], BF16, kind="ExternalInput"),
    ]
    wh_d = [
        nc.dram_tensor(f"wh{l}", [H, G4], BF16, kind="ExternalInput")
        for l in range(L)
    ]
    b_d = (
        [nc.dram_tensor(f"b{l}", [1, G4], BF16, kind="ExternalInput") for l in range(L)]
        if include_bias
        else None
    )
    # final h of the top layer back to the host (transposed bf16 layout)
    out_d = nc.dram_tensor("hout", [P, H], BF16, kind="ExternalOutput")

    with tile.TileContext(nc) as tc:
        with (
            tc.tile_pool(name="wpool", bufs=1) as wp,
            tc.tile_pool(name="state", bufs=1) as st,
            tc.tile_pool(name="work", bufs=1) as wk,
            tc.tile_pool(name="psg", bufs=2, space="PSUM") as psg,
        ):
            # ---- persistent tiles -------------------------------------------------
            identf = wp.tile([P, P], F32)
            make_identity(nc, identf)
            ident = wp.tile([P, P], BF16)
            nc.scalar.copy(ident[:], identf[:])

            xT_t = wp.tile([P, (T // 2) * P], BF16)
            nc.sync.dma_start(xT_t[:], xT_d[:])

            # Wx0 duplicated into both partition halves so odd-t x tiles
            # (living at base partition 64) find it on matching partitions.
            wx0_t = wp.tile([P, G4], BF16)
            nc.sync.dma_start(wx0_t[:IN, :], wx_d[0][:])
            nc.sync.dma_start(wx0_t[IN:, :], wx_d[0][:])
            # [H, G4] weights as [128, KH, G4]: partition = k % 128, k-chunk = k // 128
            big_w = {}
            for name, d in (
                ("wh0", wh_d[0]),
                ("wx1", wx_d[1]),
                ("wh1", wh_d[1]),
                ("wx2", wx_d[2]),
                ("wh2", wh_d[2]),
            ):
                w_t = wp.tile([P, KH, G4], BF16, name=f"{name}_t")
                nc.sync.dma_start(w_t[:], d.rearrange("(ko ki) n -> ki ko n", ki=P))
                big_w[name] = w_t

            if include_bias:
                ones_f = wp.tile([1, P], F32)
                nc.vector.memset(ones_f[:], 1.0)
                ones_t = wp.tile([1, P], BF16)
                nc.scalar.copy(ones_t[:], ones_f[:])
                b_t = []
                for l in range(L):
                    bt = wp.tile([1, G4], BF16, name=f"b{l}_t")
                    nc.sync.dma_start(bt[:], b_d[l][:])
                    b_t.append(bt)

            # states: h transposed (feature-major), C batch-major; all bf16.
            zbuf = wk.tile([P, H], F32, tag="zb")
            nc.vector.memset(zbuf[:], 0.0)
            hT = []
            Cs = []
            for l in range(L):
                h_t = st.tile([P, H], BF16, name=f"hT{l}")
                nc.scalar.copy(h_t[:], zbuf[:])
                hT.append(h_t)
                c_t = st.tile([P, H], BF16, name=f"C{l}")
                nc.scalar.copy(c_t[:], zbuf[:])
                Cs.append(c_t)

            pending_finish = [None]

            def cell(l: int, xin):
                """One LSTM cell update: xin = (lhsT, rhs, nk) of the fresh input.

                Gate bank layout (host-reordered weight columns):
                bank0 = f, bank1 = i, bank2 = o, bank3 = c.
                The previous cell's transposes + hT copy ("finish") are deferred
                to the end of this cell's emission.
                """
                if pending_finish[0] is not None and pending_finish[0][0] == l:
                    pending_finish[0][1]()
                    pending_finish[0] = None
                # Per-bank PSUM tiles: the slot release is tile-granular, so
                # splitting the 4 gate banks into 4 tiles lets the next
                # occupant's matmuls start as soon as each bank's own readers
                # finish. Bank 0 (which also hosts the previous occupant's
                # h-transposes + hT copy) is written LAST in each k-group so
                # its release hides under 3 matmuls.
                gb = [
                    psg.tile([P, 512], F32, name=f"g{n}", tag=f"g{n}")
                    for n in range(NB)
                ]
                if include_bias:
                    for n in range(NB):
                        nc.tensor.matmul(
                            gb[n][:],
                            ones_t[:],
                            b_t[l][:, ts(n, 512)],
                            start=True,
                            stop=False,
                        )
                # Banks 1-3 first (state then input, k-outer); bank 0 entirely
                # last so its release (which includes the previous occupant's
                # transposes + hT copy) hides under ~24 matmuls, and so the
                # c/i/o gates complete early enough for the tail to overlap
                # this cell's own bank-0 matmuls.
                nk = len(xin)
                for k in range(KH):
                    lhsT = hT[l][:, ts(k, P)]
                    rhs = big_w[f"wh{l}"][:, k]
                    for n in (1, 2, 3):
                        nc.tensor.matmul(
                            gb[n][:],
                            lhsT,
                            rhs[:, ts(n, 512)],
                            start=(k == 0 and not include_bias),
                            stop=False,
                            skip_group_check=True,
                        )
                for kidx, (lhsT, rhs) in enumerate(xin):
                    for n in (1, 2, 3):
                        nc.tensor.matmul(
                            gb[n][:],
                            lhsT,
                            rhs[:, ts(n, 512)],
                            start=False,
                            stop=(kidx == nk - 1),
                            skip_group_check=True,
                        )
                for k in range(KH):
                    nc.tensor.matmul(
                        gb[0][:],
                        hT[l][:, ts(k, P)],
                        big_w[f"wh{l}"][:, k][:, ts(0, 512)],
                        start=(k == 0 and not include_bias),
                        stop=False,
                        skip_group_check=True,
                    )
                for kidx, (lhsT, rhs) in enumerate(xin):
                    nc.tensor.matmul(
                        gb[0][:],
                        lhsT,
                        rhs[:, ts(0, 512)],
                        start=False,
                        stop=(kidx == nk - 1),
                        skip_group_check=True,
                    )

                # Flush the previous cell's finish now — after this cell's
                # matmuls (so every emitted reader of the previous hT version
                # precedes the new write) but BEFORE this cell's gate-tail DVE
                # ops, so the hT copy doesn't queue behind them in DVE's FIFO.
                if pending_finish[0] is not None:
                    pending_finish[0][1]()
                    pending_finish[0] = None

                fio_s = wk.tile([P, 3 * H], BF16)
                c_s = wk.tile([P, H], BF16)
                tanC = wk.tile([P, H], BF16)
                h_b = wk.tile([P, H], BF16)
                fC = wk.tile([P, H], BF16)
                ic = wk.tile([P, H], BF16)

                # gates: bank0=f, bank1=i, bank2=o sigmoid (per-bank tiles =>
                # per-bank ACTs); bank3 = c tanh. Ordered so the DVE chain's
                # inputs (c, i, f) land first.
                f_s = fio_s[:, 0:H]
                i_s = fio_s[:, H : 2 * H]
                o_s = fio_s[:, 2 * H : 3 * H]
                nc.scalar.activation(c_s[:], gb[3][:], AF.Tanh)
                nc.scalar.activation(i_s, gb[1][:], AF.Sigmoid)
                nc.scalar.activation(o_s, gb[2][:], AF.Sigmoid)
                nc.scalar.activation(f_s, gb[0][:], AF.Sigmoid)
                nc.vector.tensor_mul(ic[:], i_s, c_s[:])
                nc.vector.tensor_mul(fC[:], f_s, Cs[l][:])
                nc.vector.tensor_add(Cs[l][:], fC[:], ic[:])
                nc.scalar.activation(tanC[:], Cs[l][:], AF.Tanh)
                nc.vector.tensor_mul(h_b[:], o_s, tanC[:])

                # h -> transposed bf16 state via PE transpose into this cell's
                # own g bank 0 (f gate already consumed), deferred to
                # interleave with the next cell's matmuls.
                g0b = gb[0][:].bitcast(BF16)  # [P, 1024] bf16 view of bank 0

                def finish(l=l, g0b=g0b, h_b=h_b):
                    for j in range(KH):
                        nc.tensor.transpose(
                            g0b[:, ts(j, P)], h_b[:, ts(j, P)], ident[:]
                        )
                    nc.vector.tensor_copy(hT[l][:], g0b[:, :H])

                pending_finish[0] = (l, finish)

            def emit_cell(t: int, l: int):
                if l == 0:
                    r0 = 0 if t % 2 == 0 else 64
                    xin = [(xT_t[r0 : r0 + IN, ts(t // 2, P)], wx0_t[r0 : r0 + IN, :])]
                else:
                    xin = [
                        (hT[l - 1][:, ts(j, P)], big_w[f"wx{l}"][:, j])
                        for j in range(KH)
                    ]
                cell(l, xin)

            def whole_pass():
                # wavefront order: cells (s,0), (s-1,1), (s-2,2)
                for s in range(T + L - 1):
                    for l in range(L):
                        t = s - l
                        if 0 <= t < T:
                            emit_cell(t, l)
                if pending_finish[0] is not None:
                    pending_finish[0][1]()
                    pending_finish[0] = None

            if reps > 1:
                with tc.For_i(0, reps, 1):
                    whole_pass()
            else:
                whole_pass()

            if pending_finish[0] is not None:
                pending_finish[0][1]()
                pending_finish[0] = None

            # ---- ship final top-layer h back to the host --------------------------
            nc.sync.dma_start(out_d[:], hT[L - 1][:])

    nc.finalize()
    return nc


_NC_CACHE: dict = {}
_LAST_RUN: dict = {}

# host-side gate reorder [f,i,c,o] -> [f,i,o,c]
_PERM = np.concatenate(
    [
        np.arange(0, H),          # f
        np.arange(H, 2 * H),      # i
        np.arange(3 * H, 4 * H),  # o
        np.arange(2 * H, 3 * H),  # c
    ]
)


def _pack_xT(x_shard: np.ndarray) -> np.ndarray:
    """[128, T, IN] -> [128, (T//2)*128] packed transposed layout (bf16 u16)."""
    xt = np.zeros((P, (T // 2) * P), dtype=np.float32)
    for t in range(T):
        r0 = 0 if t % 2 == 0 else 64
        xt[r0 : r0 + IN, (t // 2) * P : (t // 2 + 1) * P] = x_shard[:, t, :].T
    return _to_bf16(xt)


def kernel(**inputs) -> np.ndarray:
    x = np.ascontiguousarray(np.asarray(inputs["x"], dtype=np.float32))
    B = x.shape[0]
    assert B % NCORES == 0
    Bl = B // NCORES

    ws = {}
    for name in ("Wx0", "Wh0", "Wx1", "Wh1", "Wx2", "Wh2"):
        w = np.asarray(inputs[name], dtype=np.float32)[:, _PERM]
        ws[name] = _to_bf16(w)
    fc_w = np.asarray(inputs["fc_w"], dtype=np.float32)
    bs = [np.asarray(inputs[f"b{l}"], dtype=np.float32)[_PERM] for l in range(L)]
    fc_b = np.asarray(inputs["fc_b"], dtype=np.float32)
    include_bias = any(np.any(b != 0) for b in bs)

    key = include_bias
    if key not in _NC_CACHE:
        _NC_CACHE[key] = _build(include_bias)
    nc = _NC_CACHE[key]
    _LAST_RUN["include_bias"] = include_bias

    in_maps = []
    for c in range(NCORES):
        m = {
            "xT": _pack_xT(x[c * Bl : (c + 1) * Bl]),
            "wx0": ws["Wx0"],
            "wx1": ws["Wx1"],
            "wx2": ws["Wx2"],
            "wh0": ws["Wh0"],
            "wh1": ws["Wh1"],
            "wh2": ws["Wh2"],
        }
        if include_bias:
            for l in range(L):
                m[f"b{l}"] = _to_bf16(bs[l]).reshape(1, G4)
        in_maps.append(m)

    res = bass_utils.run_bass_kernel_spmd(nc, in_maps, core_ids=list(range(NCORES)))
    _LAST_RUN["nc"] = nc
    _LAST_RUN["in_maps"] = in_maps
    outs = []
    for c in range(NCORES):
        hu = res.results[c]["hout"]  # [128, 512] bf16-as-u16 (or float)
        hu = np.asarray(hu)
        if hu.dtype == np.uint16:
            ht = (hu.astype(np.uint32) << 16).view(np.float32)
        else:
            ht = hu.astype(np.float32)
        # ht[p, 128*j + b] = h2[b, 128*j + p]
        h2 = ht.reshape(P, KH, P).transpose(2, 1, 0).reshape(P, H)
        outs.append(h2 @ fc_w)
    out = np.concatenate(outs, axis=0)
    return (out + fc_b.reshape(1, -1)).astype(np.float32)

